# revision 2
# baseline (speedup 1.0000x reference)
"""Trainium2 Bass kernel for nn_ContrastiveLoss (NT-Xent with sampled negatives).

Reference semantics (B=4096, D=512, N=8192, R=4 negatives/row, temp=0.5+1e-8):
    z  = concat(z_i, z_j)                       [N, D]
    zn = z / max(||z||, 1e-8)
    sim = (zn @ zn.T) / temp
    pos[i]  = sim[i, (i+B) % N]
    cols    = neg_idx + (neg_idx >= row)        (skip-diagonal remap)
    neg[i,k] = sim[i, cols[i,k]]
    nll = logsumexp([pos, neg]) - pos ;  loss = mean(nll)

Sharding: positive-pair data parallel. Core m owns the 512 i-rows
[m*512, (m+1)*512) AND their 512 positive partners [B + m*512, ...), so each
positive dot is computed once and shared by both rows of the pair, and the
partner-row norms coincide with own-row norms (no separate partner-norm pass).

Per core: load own 1024 rows (bf16, host-staged), batch-gather the 4096
negative rows with per-j dma_gather (Pool), compute dots with DVE 2x bf16
multiplies + 4x tensor_scalar fused accumulate, norms split across
ACT Square+accum / Pool scalar_tensor_tensor / DVE, inverse norms as
exp(-0.5*ln(ss)) (single activation table), 5-wide log-softmax, per-partition
partial sums; host adds the 8x128 partials.
"""

import sys

import numpy as np

if "/opt/trn_rl_repo" not in sys.path:
    sys.path.insert(0, "/opt/trn_rl_repo")

import ml_dtypes

B = 4096
D = 512
N = 2 * B
R = 4  # negatives per row
NCORES = 8
SPC = B // NCORES  # i-rows per core = 512
RPC = 2 * SPC  # local rows per core = 1024 (i-rows + partner rows)
P = 128  # partitions
J = RPC // P  # row-tiles per core = 8
JH = J // 2  # 4: j<JH are i-rows, j>=JH are partner rows
TEMP = 0.5 + 1e-08
INV_TEMP = float(1.0 / TEMP)
NLE_TABLE = 6  # act_func_sets index of natural_log_exp_and_others

_CACHE = {}


# negative-dot chunks per j: j0..5 DVE (streamed), j6..7 Pool (post-stream)
def _dot_engine(j):
    return "dve" if j < 6 else "pool"


# gathered-row square chunk (j, k) -> engine (18 ACT, 6 DVE, 8 Pool)
def _gsq_engine(j, k):
    if j >= 6:
        return "pool"
    return "dve" if k == 3 else "act"


def build_nc():
    import concourse.bass as bass
    import concourse.bacc as bacc
    import concourse.mybir as mybir
    from concourse.tile import TileContext
    from concourse.tile_rust import add_dep_helper

    fp32 = mybir.dt.float32
    bf16 = mybir.dt.bfloat16
    i16 = mybir.dt.int16
    AF = mybir.ActivationFunctionType
    OP = mybir.AluOpType

    nc = bacc.Bacc()
    zb = nc.dram_tensor("zb", [N, D], bf16, kind="ExternalInput")
    zop = nc.dram_tensor("zop", [RPC, D], bf16, kind="ExternalInput")
    gidx = nc.dram_tensor("gidx", [P, J, 32], i16, kind="ExternalInput")
    out_partial = nc.dram_tensor("partial", [P, 1], fp32, kind="ExternalOutput")

    with TileContext(nc) as tc:
        with (
            tc.tile_pool(name="big", bufs=1) as big,
            tc.tile_pool(name="small", bufs=1) as small,
            tc.tile_pool(name="scr", bufs=4) as scr,
        ):
            # ---- manual activation-table load: natural_log_exp_and_others
            # covers Square/Ln/Exp, so this is the ONLY table load and it
            # overlaps the DMA lead-in.
            atl = mybir.InstLoadActFuncSet(
                name=nc.get_next_instruction_name(), ins=[], outs=[],
                act_func_set_id=NLE_TABLE,
            )
            atl.engine = mybir.EngineType.Activation
            nc.scalar.add_instruction(atl)

            # ---- constants ----
            sc_half = small.tile([P, 1], fp32, tag="sc_half")  # -0.5
            bi_temp = small.tile([P, 1], fp32, tag="bi_temp")  # ln(1/temp)
            nc.vector.memset(sc_half[:], -0.5)
            nc.vector.memset(bi_temp[:], float(np.log(INV_TEMP)))

            # ---- input DMAs (SP queue) ----
            idxs = small.tile([P, J, 32], i16, tag="idxs")
            nc.sync.dma_start(out=idxs[:], in_=gidx[:])

            # own rows as 4 pair-chunk DMAs: DMA i brings j=i and j=i+4 so the
            # positive pair (i, i+4) completes as early as possible.
            a_t = big.tile([P, J, D], bf16, tag="A")
            zop_v = zop[:].rearrange("(j p) d -> p j d", p=P)
            for i in range(JH):
                nc.sync.dma_start(
                    out=a_t[:, i : i + JH + 1 : JH, :],
                    in_=zop_v[:, i : i + JH + 1 : JH, :],
                )

            # ---- negative-row gathers: one dma_gather per j (Pool) ----
            g_t = big.tile([P, J, R, D], bf16, tag="G")
            for j in range(J):
                nc.gpsimd.dma_gather(
                    out_ap=g_t[:, j, :, :],
                    in_ap=zb[:],
                    idxs_ap=idxs[:, j, :],
                    num_idxs=R * P,
                    num_idxs_reg=R * P,
                    elem_size=D,
                )

            # ---- accumulator tiles ----
            dots = small.tile([P, J, 1 + R], fp32, tag="dots")
            ssa = small.tile([P, J, 1], fp32, tag="ssa")
            ssg = small.tile([P, J, R], fp32, tag="ssg")
            dumpv = big.tile([P, D], bf16, tag="dumpv")  # TSP out dump (DVE)
            dumpp = big.tile([P, D], bf16, tag="dumpp")  # STT out dump (Pool)
            dumpa = big.tile([P, D], bf16, tag="dumpa")  # ACT square out dump

            prev_dve = [None]

            def dve(inst):
                # chain DVE order so each op carries at most one new sem wait
                if prev_dve[0] is not None:
                    add_dep_helper(inst.ins, prev_dve[0].ins, sync=False,
                                   reason="dve-order")
                prev_dve[0] = inst
                return inst

            def tsp_reduce(src_ap, accum_ap):
                # accum_ap [P,1] (fp32) = sum(src_ap [P,D] bf16); 4x DVE mode
                return dve(nc.vector.tensor_scalar(
                    out=dumpv[:], in0=src_ap, scalar1=1.0, scalar2=None,
                    op0=OP.mult, op1=OP.add, accum_out=accum_ap,
                ))

            def pool_fused(in0_ap, in1_ap, accum_ap):
                # accum_ap [P,1] = sum(in0*in1) on Pool
                nc.gpsimd.scalar_tensor_tensor(
                    out=dumpp[:], in0=in0_ap, scalar=1.0, in1=in1_ap,
                    op0=OP.mult, op1=OP.mult, accum_out=accum_ap,
                )

            def act_sq(in_ap, accum_ap):
                nc.scalar.activation(
                    out=dumpa[:], in_=in_ap, func=AF.Square, accum_out=accum_ap,
                )

            # ================= ACT stream (emission order = exec order) ====
            # own-row squares j=4..7 first (data lands early)
            for j in range(JH, J):
                act_sq(a_t[:, j, :], ssa[:, j, :])

            # ================= DVE early block ============================
            # positive dots (pairs (i, i+4), computed once)
            for h in range(2):
                js = slice(2 * h, 2 * h + 2)
                prod = scr.tile([P, 2, D], bf16, tag="prodpos")
                dve(nc.vector.tensor_tensor(
                    out=prod[:], in0=a_t[:, js, :],
                    in1=a_t[:, 2 * h + JH : 2 * h + JH + 2, :], op=OP.mult,
                ))
                for i in range(2):
                    tsp_reduce(prod[:, i, :], dots[:, 2 * h + i, 0:1])
            # own-row squares j=0..3 on DVE
            for h in range(2):
                js = slice(2 * h, 2 * h + 2)
                prod = scr.tile([P, 2, D], bf16, tag="prodosq")
                dve(nc.vector.tensor_tensor(
                    out=prod[:], in0=a_t[:, js, :], in1=a_t[:, js, :],
                    op=OP.mult,
                ))
                for i in range(2):
                    tsp_reduce(prod[:, i, :], ssa[:, 2 * h + i, :])
            # ---- early ACT asm: row inverse norms (ssa complete by now;
            # ACT-only ops so the DVE order chain is not stalled) ----
            lna = small.tile([P, J, 1], fp32, tag="lna")
            nc.scalar.activation(out=lna[:], in_=ssa[:], func=AF.Ln)
            invp = small.tile([P, J, 1], fp32, tag="invp")  # rsqrt(ssa)
            nc.scalar.activation(out=invp[:], in_=lna[:], func=AF.Exp,
                                 scale=sc_half[:])
            invt = small.tile([P, J, 1], fp32, tag="invt")  # rsqrt(ssa)/temp
            nc.scalar.activation(out=invt[:], in_=lna[:], func=AF.Exp,
                                 scale=sc_half[:], bias=bi_temp[:])

            # ============== per-j streamed work ===========================
            for j in range(J):
                # DVE: negative dots (wide mult + 4 fused reduces)
                if _dot_engine(j) == "dve":
                    prod = scr.tile([P, R, D], bf16, tag="prodneg")
                    dve(nc.vector.tensor_tensor(
                        out=prod[:],
                        in0=a_t[:, j : j + 1, :].to_broadcast([P, R, D]),
                        in1=g_t[:, j, :, :],
                        op=OP.mult,
                    ))
                    for k in range(R):
                        tsp_reduce(prod[:, k, :], dots[:, j, 1 + k : 2 + k])
                # ACT squares for this j
                for k in range(R):
                    if _gsq_engine(j, k) == "act":
                        act_sq(g_t[:, j, k, :], ssg[:, j, k : k + 1])
                # DVE squares for this j
                dve_ks = [k for k in range(R) if _gsq_engine(j, k) == "dve"]
                if dve_ks:
                    k0, k1 = dve_ks[0], dve_ks[-1] + 1
                    nk = k1 - k0
                    prod = scr.tile([P, nk, D], bf16, tag="prodgsq")
                    dve(nc.vector.tensor_tensor(
                        out=prod[:], in0=g_t[:, j, k0:k1, :],
                        in1=g_t[:, j, k0:k1, :], op=OP.mult,
                    ))
                    for k in range(k0, k1):
                        tsp_reduce(prod[:, k - k0, :], ssg[:, j, k : k + 1])

            # ---- Pool post-gather-stream: squares first, then dots so the
            # final tail (logits -> nll) only waits on the dot chain ----
            for j in range(J):
                for k in range(R):
                    if _gsq_engine(j, k) == "pool":
                        pool_fused(g_t[:, j, k, :], g_t[:, j, k, :],
                                   ssg[:, j, k : k + 1])
            for j in range(J):
                if _dot_engine(j) == "pool":
                    for k in range(R):
                        pool_fused(a_t[:, j, :], g_t[:, j, k, :],
                                   dots[:, j, 1 + k : 2 + k])

            # ---- gathered-row inverse norms (needs all ssg) ----
            lng = small.tile([P, J, R], fp32, tag="lng")
            nc.scalar.activation(out=lng[:], in_=ssg[:], func=AF.Ln)
            invg = small.tile([P, J, R], fp32, tag="invg")  # rsqrt(ssg)/temp
            nc.scalar.activation(out=invg[:], in_=lng[:], func=AF.Exp,
                                 scale=sc_half[:], bias=bi_temp[:])

            # positive dots shared with partner rows; partner-side inv norms
            dve(nc.vector.tensor_copy(
                out=dots[:, JH:J, 0:1], in_=dots[:, 0:JH, 0:1]))
            pinv = small.tile([P, J, 1 + R], fp32, tag="pinv")
            dve(nc.vector.tensor_copy(out=pinv[:, 0:JH, 0:1],
                                      in_=invt[:, JH:J, :]))
            dve(nc.vector.tensor_copy(out=pinv[:, JH:J, 0:1],
                                      in_=invt[:, 0:JH, :]))
            dve(nc.vector.tensor_copy(out=pinv[:, :, 1 : 1 + R], in_=invg[:]))

            # ---- logits = dots * invp_row * pinv ----
            l_t = small.tile([P, J, 1 + R], fp32, tag="logits")
            dve(nc.vector.tensor_tensor(out=l_t[:], in0=dots[:], in1=pinv[:],
                                        op=OP.mult))
            dve(nc.vector.tensor_tensor(
                out=l_t[:], in0=l_t[:],
                in1=invp[:].to_broadcast([P, J, 1 + R]), op=OP.mult))

            # ---- nll = ln(sum(exp(l))) - l_pos  (|l| <= 2, no max-shift) ----
            e_t = small.tile([P, J, 1 + R], fp32, tag="exps")
            nc.scalar.activation(out=e_t[:], in_=l_t[:], func=AF.Exp)
            s_t = small.tile([P, J, 1], fp32, tag="sume")
            dve(nc.vector.tensor_reduce(
                out=s_t[:], in_=e_t[:], axis=mybir.AxisListType.X, op=OP.add))
            lns = small.tile([P, J, 1], fp32, tag="lns")
            nc.scalar.activation(out=lns[:], in_=s_t[:], func=AF.Ln)
            nll = small.tile([P, J, 1], fp32, tag="nll")
            dve(nc.vector.tensor_tensor(
                out=nll[:], in0=lns[:], in1=l_t[:, :, 0:1], op=OP.subtract))

            # ---- per-partition partial sums; host adds the 128x8 values ----
            rsum = small.tile([P, 1], fp32, tag="rsum")
            dve(nc.vector.tensor_reduce(
                out=rsum[:], in_=nll[:], axis=mybir.AxisListType.XY, op=OP.add))
            nc.sync.dma_start(out=out_partial[:], in_=rsum[:])

    nc.finalize()
    return nc


def make_in_maps(z_i, z_j, neg_idx):
    z = np.concatenate([np.asarray(z_i), np.asarray(z_j)], axis=0)
    zb = np.ascontiguousarray(z.astype(ml_dtypes.bfloat16))
    neg_idx = np.asarray(neg_idx, dtype=np.int64)
    rows_all = np.arange(N, dtype=np.int64)
    cols_all = neg_idx + (neg_idx >= rows_all[:, None])  # [N, R] in [0, N-1]

    in_maps = []
    for m in range(NCORES):
        lo = m * SPC
        gr = np.concatenate(
            [rows_all[lo : lo + SPC], rows_all[B + lo : B + lo + SPC]]
        )  # local row l -> global row; l = j*128 + p
        zop = np.ascontiguousarray(zb[gr])

        # gather indices: per j, slot i = k*128 + p maps to cols[gr[j*128+p], k]
        cols_loc = cols_all[gr].astype(np.int16)  # [1024, R]
        gidx = np.empty((P, J, 32), dtype=np.int16)
        for j in range(J):
            flat = cols_loc[j * P : (j + 1) * P].T.reshape(-1)  # [R*P], i=k*128+p
            wrapped = flat.reshape(32, 16).T  # [16, 32]
            gidx[:, j, :] = np.tile(wrapped, (8, 1))
        in_maps.append({"zb": zb, "zop": zop, "gidx": np.ascontiguousarray(gidx)})
    return in_maps


def kernel(z_i, z_j, neg_idx, _bench=None):
    from concourse.bass_utils import run_bass_kernel_spmd

    if "nc" not in _CACHE:
        _CACHE["nc"] = build_nc()
    nc = _CACHE["nc"]
    in_maps = make_in_maps(z_i, z_j, neg_idx)
    core_ids = list(range(NCORES))
    kw = dict(_bench or {})
    r = run_bass_kernel_spmd(nc, in_maps, core_ids, **kw)
    if _bench is not None:
        _CACHE["last_results"] = r
    total = np.sum(
        [np.asarray(r.results[m]["partial"], dtype=np.float64).sum()
         for m in range(NCORES)],
    )
    return np.float32(total / N)


# revision 3
# speedup vs baseline: 1.1134x; 1.1134x over previous
"""Trainium2 Bass kernel for nn_ContrastiveLoss (NT-Xent with sampled negatives).

Reference semantics (B=4096, D=512, N=8192, R=4 negatives/row, temp=0.5+1e-8):
    z  = concat(z_i, z_j)                       [N, D]
    zn = z / max(||z||, 1e-8)
    sim = (zn @ zn.T) / temp
    pos[i]  = sim[i, (i+B) % N]
    cols    = neg_idx + (neg_idx >= row)        (skip-diagonal remap)
    neg[i,k] = sim[i, cols[i,k]]
    nll = logsumexp([pos, neg]) - pos ;  loss = mean(nll)

Sharding: positive-pair data parallel. Core m owns the 512 i-rows
[m*512, (m+1)*512) AND their 512 positive partners [B + m*512, ...), so each
positive dot is computed once and shared by both rows of the pair, and the
partner-row norms coincide with own-row norms (no separate partner-norm pass).

Per core: load own 1024 rows (bf16, host-staged), batch-gather the 4096
negative rows with per-j dma_gather (Pool), compute dots with DVE 2x bf16
multiplies + 4x tensor_scalar fused accumulate, norms split across
ACT Square+accum / Pool scalar_tensor_tensor / DVE, inverse norms as
exp(-0.5*ln(ss)) (single activation table), 5-wide log-softmax, per-partition
partial sums; host adds the 8x128 partials.
"""

import sys

import numpy as np

if "/opt/trn_rl_repo" not in sys.path:
    sys.path.insert(0, "/opt/trn_rl_repo")

import ml_dtypes

B = 4096
D = 512
N = 2 * B
R = 4  # negatives per row
NCORES = 8
SPC = B // NCORES  # i-rows per core = 512
RPC = 2 * SPC  # local rows per core = 1024 (i-rows + partner rows)
P = 128  # partitions
J = RPC // P  # row-tiles per core = 8
JH = J // 2  # 4: j<JH are i-rows, j>=JH are partner rows
TEMP = 0.5 + 1e-08
INV_TEMP = float(1.0 / TEMP)
NLE_TABLE = 6  # act_func_sets index of natural_log_exp_and_others

_CACHE = {}


# negative-dot chunks per j: j0..5 DVE (streamed), j6..7 Pool-mult (post-stream)
def _dot_engine(j):
    return "dve" if j < 6 else "pool"


# gathered-row square chunk (j, k) -> engine (18 ACT fused, 6+8 Pool-mult)
# Pool has no HW-legal fused accumulate, so Pool only multiplies (TT) and the
# reduce runs as a DVE tensor_scalar accumulate.
def _gsq_engine(j, k):
    if j >= 6:
        return "pool"
    if k == 3:
        return "dve" if j < 2 else "pool"
    return "act"


def build_nc():
    import concourse.bass as bass
    import concourse.bacc as bacc
    import concourse.mybir as mybir
    from concourse.tile import TileContext
    from concourse.tile_rust import add_dep_helper

    fp32 = mybir.dt.float32
    bf16 = mybir.dt.bfloat16
    i16 = mybir.dt.int16
    AF = mybir.ActivationFunctionType
    OP = mybir.AluOpType

    nc = bacc.Bacc()
    zb = nc.dram_tensor("zb", [N, D], bf16, kind="ExternalInput")
    zop = nc.dram_tensor("zop", [RPC, D], bf16, kind="ExternalInput")
    gidx = nc.dram_tensor("gidx", [P, J, 32], i16, kind="ExternalInput")
    out_partial = nc.dram_tensor("partial", [P, 1], fp32, kind="ExternalOutput")

    with TileContext(nc) as tc:
        with (
            tc.tile_pool(name="big", bufs=1) as big,
            tc.tile_pool(name="small", bufs=1) as small,
            tc.tile_pool(name="scr", bufs=4) as scr,
        ):
            # ---- manual activation-table load: natural_log_exp_and_others
            # covers Square/Ln/Exp, so this is the ONLY table load and it
            # overlaps the DMA lead-in.
            atl = mybir.InstLoadActFuncSet(
                name=nc.get_next_instruction_name(), ins=[], outs=[],
                act_func_set_id=NLE_TABLE,
            )
            atl.engine = mybir.EngineType.Activation
            nc.scalar.add_instruction(atl)

            # ---- constants ----
            sc_half = small.tile([P, 1], fp32, tag="sc_half")  # -0.5
            bi_temp = small.tile([P, 1], fp32, tag="bi_temp")  # ln(1/temp)
            nc.vector.memset(sc_half[:], -0.5)
            nc.vector.memset(bi_temp[:], float(np.log(INV_TEMP)))

            # ---- input DMAs (SP queue) ----
            idxs = small.tile([P, J, 32], i16, tag="idxs")
            nc.sync.dma_start(out=idxs[:], in_=gidx[:])

            # own rows as 4 pair-chunk DMAs: DMA i brings j=i and j=i+4 so the
            # positive pair (i, i+4) completes as early as possible.
            a_t = big.tile([P, J, D], bf16, tag="A")
            zop_v = zop[:].rearrange("(j p) d -> p j d", p=P)
            for i in range(JH):
                nc.sync.dma_start(
                    out=a_t[:, i : i + JH + 1 : JH, :],
                    in_=zop_v[:, i : i + JH + 1 : JH, :],
                )

            # ---- negative-row gathers: one dma_gather per j (Pool) ----
            g_t = big.tile([P, J, R, D], bf16, tag="G")
            for j in range(J):
                nc.gpsimd.dma_gather(
                    out_ap=g_t[:, j, :, :],
                    in_ap=zb[:],
                    idxs_ap=idxs[:, j, :],
                    num_idxs=R * P,
                    num_idxs_reg=R * P,
                    elem_size=D,
                )

            # ---- accumulator tiles ----
            dots = small.tile([P, J, 1 + R], fp32, tag="dots")
            ssa = small.tile([P, J, 1], fp32, tag="ssa")
            ssg = small.tile([P, J, R], fp32, tag="ssg")
            dumpv = big.tile([P, D], bf16, tag="dumpv")  # TSP out dump (DVE)
            dumpp = big.tile([P, D], bf16, tag="dumpp")  # STT out dump (Pool)
            dumpa = big.tile([P, D], bf16, tag="dumpa")  # ACT square out dump

            prev_dve = [None]

            def dve(inst):
                # chain DVE order so each op carries at most one new sem wait
                if prev_dve[0] is not None:
                    add_dep_helper(inst.ins, prev_dve[0].ins, sync=False,
                                   reason="dve-order")
                prev_dve[0] = inst
                return inst

            def tsp_reduce(src_ap, accum_ap):
                # accum_ap [P,1] (fp32) = sum(src_ap [P,D] bf16); 4x DVE mode
                return dve(nc.vector.tensor_scalar(
                    out=dumpv[:], in0=src_ap, scalar1=1.0, scalar2=None,
                    op0=OP.mult, op1=OP.add, accum_out=accum_ap,
                ))

            # Pool multiplies into product tiles; DVE reduces them later.
            pool_red = []  # (src_ap, accum_ap) queue, in Pool completion order

            def pool_mult(out_ap, in0_ap, in1_ap):
                nc.gpsimd.tensor_tensor(
                    out=out_ap, in0=in0_ap, in1=in1_ap, op=OP.mult)

            def act_sq(in_ap, accum_ap):
                nc.scalar.activation(
                    out=dumpa[:], in_=in_ap, func=AF.Square, accum_out=accum_ap,
                )

            # ================= ACT stream (emission order = exec order) ====
            # own-row squares j=4..7 first (data lands early)
            for j in range(JH, J):
                act_sq(a_t[:, j, :], ssa[:, j, :])

            # ================= DVE early block ============================
            # positive dots (pairs (i, i+4), computed once)
            for h in range(2):
                js = slice(2 * h, 2 * h + 2)
                prod = scr.tile([P, 2, D], bf16, tag="prodpos")
                dve(nc.vector.tensor_tensor(
                    out=prod[:], in0=a_t[:, js, :],
                    in1=a_t[:, 2 * h + JH : 2 * h + JH + 2, :], op=OP.mult,
                ))
                for i in range(2):
                    tsp_reduce(prod[:, i, :], dots[:, 2 * h + i, 0:1])
            # own-row squares j=0..3 on DVE
            for h in range(2):
                js = slice(2 * h, 2 * h + 2)
                prod = scr.tile([P, 2, D], bf16, tag="prodosq")
                dve(nc.vector.tensor_tensor(
                    out=prod[:], in0=a_t[:, js, :], in1=a_t[:, js, :],
                    op=OP.mult,
                ))
                for i in range(2):
                    tsp_reduce(prod[:, i, :], ssa[:, 2 * h + i, :])
            # ---- early ACT asm: row inverse norms (ssa complete by now;
            # ACT-only ops so the DVE order chain is not stalled) ----
            lna = small.tile([P, J, 1], fp32, tag="lna")
            nc.scalar.activation(out=lna[:], in_=ssa[:], func=AF.Ln)
            invp = small.tile([P, J, 1], fp32, tag="invp")  # rsqrt(ssa)
            nc.scalar.activation(out=invp[:], in_=lna[:], func=AF.Exp,
                                 scale=sc_half[:])
            invt = small.tile([P, J, 1], fp32, tag="invt")  # rsqrt(ssa)/temp
            nc.scalar.activation(out=invt[:], in_=lna[:], func=AF.Exp,
                                 scale=sc_half[:], bias=bi_temp[:])

            # ============== per-j streamed work ===========================
            for j in range(J):
                # DVE: negative dots (wide mult + 4 fused reduces)
                if _dot_engine(j) == "dve":
                    prod = scr.tile([P, R, D], bf16, tag="prodneg")
                    dve(nc.vector.tensor_tensor(
                        out=prod[:],
                        in0=a_t[:, j : j + 1, :].to_broadcast([P, R, D]),
                        in1=g_t[:, j, :, :],
                        op=OP.mult,
                    ))
                    for k in range(R):
                        tsp_reduce(prod[:, k, :], dots[:, j, 1 + k : 2 + k])
                # ACT squares for this j
                for k in range(R):
                    if _gsq_engine(j, k) == "act":
                        act_sq(g_t[:, j, k, :], ssg[:, j, k : k + 1])
                # DVE squares for this j (narrow mult + fused accumulate)
                for k in range(R):
                    if _gsq_engine(j, k) == "dve":
                        prodq = scr.tile([P, 1, D], bf16, tag="prodq")
                        dve(nc.vector.tensor_tensor(
                            out=prodq[:], in0=g_t[:, j, k : k + 1, :],
                            in1=g_t[:, j, k : k + 1, :], op=OP.mult,
                        ))
                        tsp_reduce(prodq[:, 0, :], ssg[:, j, k : k + 1])

            # ---- Pool post-gather-stream multiplies: squares first, then
            # dots so the final tail (logits -> nll) only waits on the dot
            # chain; DVE chases with the fused accumulates ----
            pprod = big.tile([P, 22, D], bf16, tag="pprod")
            np_ = [0]

            def pool_chunk(in0_ap, in1_ap, accum_ap):
                slot = pprod[:, np_[0], :]
                np_[0] += 1
                pool_mult(slot, in0_ap, in1_ap)
                pool_red.append((slot, accum_ap))

            for j in range(J):
                for k in range(R):
                    if _gsq_engine(j, k) == "pool":
                        pool_chunk(g_t[:, j, k, :], g_t[:, j, k, :],
                                   ssg[:, j, k : k + 1])
            for j in range(J):
                if _dot_engine(j) == "pool":
                    for k in range(R):
                        pool_chunk(a_t[:, j, :], g_t[:, j, k, :],
                                   dots[:, j, 1 + k : 2 + k])
            for src_ap, accum_ap in pool_red:
                tsp_reduce(src_ap, accum_ap)

            # ---- gathered-row inverse norms (needs all ssg) ----
            lng = small.tile([P, J, R], fp32, tag="lng")
            nc.scalar.activation(out=lng[:], in_=ssg[:], func=AF.Ln)
            invg = small.tile([P, J, R], fp32, tag="invg")  # rsqrt(ssg)/temp
            nc.scalar.activation(out=invg[:], in_=lng[:], func=AF.Exp,
                                 scale=sc_half[:], bias=bi_temp[:])

            # positive dots shared with partner rows; partner-side inv norms
            dve(nc.vector.tensor_copy(
                out=dots[:, JH:J, 0:1], in_=dots[:, 0:JH, 0:1]))
            pinv = small.tile([P, J, 1 + R], fp32, tag="pinv")
            dve(nc.vector.tensor_copy(out=pinv[:, 0:JH, 0:1],
                                      in_=invt[:, JH:J, :]))
            dve(nc.vector.tensor_copy(out=pinv[:, JH:J, 0:1],
                                      in_=invt[:, 0:JH, :]))
            dve(nc.vector.tensor_copy(out=pinv[:, :, 1 : 1 + R], in_=invg[:]))

            # ---- logits = dots * invp_row * pinv ----
            l_t = small.tile([P, J, 1 + R], fp32, tag="logits")
            dve(nc.vector.tensor_tensor(out=l_t[:], in0=dots[:], in1=pinv[:],
                                        op=OP.mult))
            dve(nc.vector.tensor_tensor(
                out=l_t[:], in0=l_t[:],
                in1=invp[:].to_broadcast([P, J, 1 + R]), op=OP.mult))

            # ---- nll = ln(sum(exp(l))) - l_pos  (|l| <= 2, no max-shift) ----
            e_t = small.tile([P, J, 1 + R], fp32, tag="exps")
            nc.scalar.activation(out=e_t[:], in_=l_t[:], func=AF.Exp)
            s_t = small.tile([P, J, 1], fp32, tag="sume")
            dve(nc.vector.tensor_reduce(
                out=s_t[:], in_=e_t[:], axis=mybir.AxisListType.X, op=OP.add))
            lns = small.tile([P, J, 1], fp32, tag="lns")
            nc.scalar.activation(out=lns[:], in_=s_t[:], func=AF.Ln)
            nll = small.tile([P, J, 1], fp32, tag="nll")
            dve(nc.vector.tensor_tensor(
                out=nll[:], in0=lns[:], in1=l_t[:, :, 0:1], op=OP.subtract))

            # ---- per-partition partial sums; host adds the 128x8 values ----
            rsum = small.tile([P, 1], fp32, tag="rsum")
            dve(nc.vector.tensor_reduce(
                out=rsum[:], in_=nll[:], axis=mybir.AxisListType.XY, op=OP.add))
            nc.sync.dma_start(out=out_partial[:], in_=rsum[:])

    nc.finalize()
    return nc


def make_in_maps(z_i, z_j, neg_idx):
    z = np.concatenate([np.asarray(z_i), np.asarray(z_j)], axis=0)
    zb = np.ascontiguousarray(z.astype(ml_dtypes.bfloat16))
    neg_idx = np.asarray(neg_idx, dtype=np.int64)
    rows_all = np.arange(N, dtype=np.int64)
    cols_all = neg_idx + (neg_idx >= rows_all[:, None])  # [N, R] in [0, N-1]

    in_maps = []
    for m in range(NCORES):
        lo = m * SPC
        gr = np.concatenate(
            [rows_all[lo : lo + SPC], rows_all[B + lo : B + lo + SPC]]
        )  # local row l -> global row; l = j*128 + p
        zop = np.ascontiguousarray(zb[gr])

        # gather indices: per j, slot i = k*128 + p maps to cols[gr[j*128+p], k]
        cols_loc = cols_all[gr].astype(np.int16)  # [1024, R]
        gidx = np.empty((P, J, 32), dtype=np.int16)
        for j in range(J):
            flat = cols_loc[j * P : (j + 1) * P].T.reshape(-1)  # [R*P], i=k*128+p
            wrapped = flat.reshape(32, 16).T  # [16, 32]
            gidx[:, j, :] = np.tile(wrapped, (8, 1))
        in_maps.append({"zb": zb, "zop": zop, "gidx": np.ascontiguousarray(gidx)})
    return in_maps


def kernel(z_i, z_j, neg_idx, _bench=None):
    from concourse.bass_utils import run_bass_kernel_spmd

    if "nc" not in _CACHE:
        _CACHE["nc"] = build_nc()
    nc = _CACHE["nc"]
    in_maps = make_in_maps(z_i, z_j, neg_idx)
    core_ids = list(range(NCORES))
    kw = dict(_bench or {})
    r = run_bass_kernel_spmd(nc, in_maps, core_ids, **kw)
    if _bench is not None:
        _CACHE["last_results"] = r
    total = np.sum(
        [np.asarray(r.results[m]["partial"], dtype=np.float64).sum()
         for m in range(NCORES)],
    )
    return np.float32(total / N)
